# revision 20
# baseline (speedup 1.0000x reference)
"""Trainium2 Bass kernel for nn_Connector_77738908057780 (dense_mlp).

Computation (see reference):
  x   = image_features                      [B, N, H]    bf16
  f1  = mean(hidden[0:13],  axis=0)         [B, N, H]
  f2  = mean(hidden[13:26], axis=0)         [B, N, H]
  cat = concat([x, f1, f2], -1)             [B, N, 3H]
  h   = gelu(cat @ W1.T + b1)               W1 = nf4_dequant(codes1, scales1) [H, 3H]
  fg  = h @ W2.T + b2                       W2 = nf4_dequant(codes2, scales2) [H, H]
  out = w * LN(fg) + (1-w) * LN(x),         w = sigmoid(alpha)

Sharding: data-parallel over batch B=8 -> one batch element per NeuronCore.

v2 design (vs the 286us baseline):
  - skewed software pipeline: supertile st's LN/gate/store stage is emitted
    during st+1 so the DVE never idles waiting on GEMM2.
  - hidden streamed as 13 layer-PAIR DMAs per supertile (1.18 MB each), all
    on the sync HWDGE queue; transposes/weights/consts ride the scalar queue
    so neither blocks the other (separate FIFO rows, SDMA round-robins).
  - the 26-layer sums are split DVE (fast chains) + GpSimd (leading pair
    folds) so DVE load drops from ~155us to ~110us and the DMA never stalls
    on a consumer.
  - GEMM1 is k-eager: all 9 m-tiles accumulate in 4.5 PSUM banks and the
    per-k matmuls fire as soon as each cat^T k-group lands (x first, then
    f1, then f2) - keeps the PE warm (HAM throttle) and off the tail.
  - b2 is folded into GEMM2 as a 10th k-tile (all-ones stationary column x
    a [b2; 0...] row block), killing the DVE bias adds.
  - NF4 dequant of the (small, replicated) weights is host-side weight prep.
"""

import os
import sys

import numpy as np
import ml_dtypes

for _p in ("/opt/trn_rl_repo", "/root/.axon_site/_ro/trn_rl_repo"):
    if os.path.isdir(_p) and _p not in sys.path:
        sys.path.insert(0, _p)

import concourse.bass as bass
import concourse.mybir as mybir
import concourse.tile as tile
from concourse import bacc
from concourse import bass_utils

BF16 = mybir.dt.bfloat16
F32 = mybir.dt.float32
AF = mybir.ActivationFunctionType
ALU = mybir.AluOpType

NP_BF16 = ml_dtypes.bfloat16

P = 128
H = 1152
H3 = 3456
NT = 729          # tokens per core (N); B=8 cores
L = 26
KO1 = H3 // P     # 27 k-tiles for GEMM1
KO2 = H // P      # 9 k-tiles for GEMM2 (+1 ones-tile for the b2 fold)
MO = H // P       # 9 output-feature tiles
EPS = 1e-5
NCHUNK = 3        # fg free-dim chunks of 384
CH = H // NCHUNK  # 384

# Supertiles of exactly 256 tokens; the last overlaps the previous by 39
# tokens (473..511 computed twice, identical values stored twice) so that
# every DMA/compute tile is a full 128-partition tile (729 is not a
# multiple of 128; partial-partition tiles hit HW-hostile DMA paths).
SUPERTILES = [0, 256, 473]
TSUP = 256        # tokens per supertile
NSUB = 2          # 128-token subtiles per supertile

NF4_CODEBOOK = np.array([
    -1.0, -0.6961928009986877, -0.5250730514526367, -0.39491748809814453,
    -0.28444138169288635, -0.18477343022823334, -0.09105003625154495, 0.0,
    0.07958029955625534, 0.16093020141124725, 0.24611230194568634,
    0.33791524171829224, 0.4407098591327667, 0.5626170039176941,
    0.7229568362236023, 1.0], dtype=np.float32)

BLOCK = 64


def _dequant_nf4(codes, scales):
    """Match reference: codebook lookup * per-64-block absmax, cast bf16."""
    out_f, in_f = codes.shape
    w = NF4_CODEBOOK[codes].reshape(out_f, in_f // BLOCK, BLOCK)
    w = w * scales[:, :, None].astype(np.float32)
    return w.reshape(out_f, in_f)  # float32 (caller casts)


def _build_program(act=AF.Gelu):
    nc = bacc.Bacc(
        "TRN2",
        target_bir_lowering=False,
        debug=False,
        num_devices=1,
    )
    x_d = nc.dram_tensor("x", (NT, H), BF16, kind="ExternalInput").ap()
    hid_d = nc.dram_tensor("hid", (L, NT, H), BF16, kind="ExternalInput").ap()
    w1t_d = nc.dram_tensor("w1t", (H3, H), BF16, kind="ExternalInput").ap()
    w2t_d = nc.dram_tensor("w2t", ((KO2 + 1) * P, H), BF16,
                           kind="ExternalInput").ap()
    b1s_d = nc.dram_tensor("b1s", (P, MO), F32, kind="ExternalInput").ap()
    g1b_d = nc.dram_tensor("g1b", (P, H), BF16, kind="ExternalInput").ap()
    g2b_d = nc.dram_tensor("g2b", (P, H), BF16, kind="ExternalInput").ap()
    bcb_d = nc.dram_tensor("bcb", (P, H), BF16, kind="ExternalInput").ap()
    out_d = nc.dram_tensor("out", (NT, H), BF16, kind="ExternalOutput").ap()

    with tile.TileContext(nc) as tc:
        _program(nc, tc, x_d, hid_d, w1t_d, w2t_d, b1s_d,
                 g1b_d, g2b_d, bcb_d, out_d, act)

    nc.compile()
    return nc


def _program(nc, tc, x_d, hid_d, w1t_d, w2t_d, b1s_d, g1b_d, g2b_d,
             bcb_d, out_d, act=AF.Gelu):
    with (
        tc.tile_pool(name="consts", bufs=1) as cpool,
        tc.tile_pool(name="hp", bufs=8) as hpool,
        tc.tile_pool(name="acc", bufs=4) as apool,
        tc.tile_pool(name="catx", bufs=2) as cxpool,
        tc.tile_pool(name="catf", bufs=1) as cfpool,
        tc.tile_pool(name="gt", bufs=1) as gpool,
        tc.tile_pool(name="xn", bufs=2) as xpool,
        tc.tile_pool(name="fg", bufs=4) as fgpool,
        tc.tile_pool(name="outp", bufs=2) as opool,
        tc.tile_pool(name="stats", bufs=2) as spool,
        tc.tile_pool(name="tmp", bufs=1) as tpool,
        tc.tile_pool(name="psA", bufs=6, space="PSUM") as psapool,
        tc.tile_pool(name="ps2", bufs=2, space="PSUM") as ps2pool,
    ):
        # ---- constants ----
        ones_sb = cpool.tile([P, TSUP], BF16)
        nc.gpsimd.memset(ones_sb[:, :], 1.0)
        b1_sb = cpool.tile([P, MO], F32)
        g1b_sb = cpool.tile([P, H], BF16)
        g2b_sb = cpool.tile([P, H], BF16)
        bcb_sb = cpool.tile([P, H], BF16)
        w1t_sb = cpool.tile([P, KO1, H], BF16)
        w2t_sb = cpool.tile([P, KO2 + 1, H], BF16)

        w1t_r = w1t_d.rearrange("(ko p) n -> p ko n", p=P)
        w2t_r = w2t_d.rearrange("(ko p) n -> p ko n", p=P)

        # scalar (ACT) queue: small consts, then x-transposes (emitted in
        # the loop), then the weight chunks.
        nc.scalar.dma_start(b1_sb, b1s_d)
        nc.scalar.dma_start(g1b_sb, g1b_d)
        nc.scalar.dma_start(g2b_sb, g2b_d)
        nc.scalar.dma_start(bcb_sb, bcb_d)

        # state carried between supertiles for the skewed tail stage
        prev = None

        def emit_tail_a(pv):
            """Skewed LN/gate stage A: rsqrt + the normalize chains that do
            not need the final combine. gpsimd does the x side, DVE the fg
            side; ACT does the sqrt (before this supertile's Gelu table)."""
            (p_x, p_fgs, p_agg, p_rpack, p_t0) = pv
            ig = spool.tile([P, 2 * NSUB], F32, tag="ig")
            nc.vector.reciprocal(ig, p_rpack)
            nc.scalar.activation(ig, ig, AF.Sqrt)
            tmp1s, tmp2s = [], []
            for tt in range(NSUB):
                # (the scalar-ptr STT opcode only exists on DVE, not Pool)
                # tmp1 = (x - mu1) * G1 ; tmp1 = tmp1 * ig1 + Bc
                tmp1 = tpool.tile([P, H], BF16, tag="tmp1")
                nc.vector.scalar_tensor_tensor(
                    tmp1, p_x[:, tt, :], p_agg[:, tt, 0:1], g1b_sb,
                    ALU.subtract, ALU.mult)
                nc.vector.scalar_tensor_tensor(
                    tmp1, tmp1, ig[:, 2 * tt:2 * tt + 1], bcb_sb,
                    ALU.mult, ALU.add)
                tmp1s.append(tmp1)
                # tmp2 = (fg - mu2) * G2
                tmp2 = tpool.tile([P, H], BF16, tag="tmp2")
                nc.vector.scalar_tensor_tensor(
                    tmp2, p_fgs[tt], p_agg[:, tt, 2:3], g2b_sb,
                    ALU.subtract, ALU.mult)
                tmp2s.append(tmp2)
            return ig, tmp1s, tmp2s

        def emit_tail_b(ig, tmp1s, tmp2s):
            """Skewed LN/gate stage B: out = tmp2 * ig2 + tmp1 (DVE)."""
            outs = []
            for tt in range(NSUB):
                out_t = opool.tile([P, H], BF16, tag="outt")
                nc.vector.scalar_tensor_tensor(
                    out_t, tmp2s[tt], ig[:, 2 * tt + 1:2 * tt + 2], tmp1s[tt],
                    ALU.mult, ALU.add)
                outs.append(out_t)
            return outs

        for st_idx, t0 in enumerate(SUPERTILES):
            # ---- DMA issue: x + 26 per-layer loads split across queues ----
            # SP gets x + layers 0-6/13-19 (the early half of each chain);
            # ACT gets layers 7-12/20-25 plus the transposes, so neither
            # queue's in-order sequencer blocks the other's loads.
            x_nat = xpool.tile([P, NSUB, H], BF16, tag="xnat")
            nc.sync.dma_start(
                x_nat,
                x_d[t0:t0 + TSUP, :].rearrange("(s p) f -> p s f", p=P),
            )
            catx = cxpool.tile([P, NSUB, MO, P], BF16, tag="catx")
            for tt in range(NSUB):
                nc.scalar.dma_start_transpose(catx[:, tt, :, :],
                                              x_nat[:, tt, :])

            def load_layer(li, eng):
                lt = hpool.tile([P, NSUB, H], BF16, name=f"hl{li}", tag="hp")
                eng.dma_start(
                    lt,
                    hid_d[li, t0:t0 + TSUP, :].rearrange(
                        "(s p) f -> p s f", p=P))
                return lt

            lay = [None] * L
            sp_layers = list(range(0, 7)) + list(range(13, 20))
            act_layers = list(range(7, 13)) + list(range(20, 26))
            for li in sp_layers:
                lay[li] = load_layer(li, nc.sync)
            if st_idx == 0:
                # weight chunks interleave with the ACT-queue layer loads
                nc.scalar.dma_start(w1t_sb[:, 0:9, :], w1t_r[:, 0:9, :])
            for j, li in enumerate(act_layers):
                lay[li] = load_layer(li, nc.scalar)
                if st_idx == 0 and j in (2, 4, 6):
                    c0 = 9 * (j // 2)
                    if c0 < KO1:
                        nc.scalar.dma_start(w1t_sb[:, c0:c0 + 9, :],
                                            w1t_r[:, c0:c0 + 9, :])
                    else:
                        nc.scalar.dma_start(w2t_sb, w2t_r)

            # ---- skewed tail stage A for the previous supertile ----
            if prev is not None:
                p_tail = emit_tail_a(prev)

            # ---- DVE: x LN stats early (x lands first) ----
            agg = spool.tile([P, NSUB, 4], F32, tag="agg")
            rpack = spool.tile([P, 2 * NSUB], F32, tag="rpack")
            for tt in range(NSUB):
                bnx = spool.tile([P, 3, 6], F32, tag="bnx")
                for c in range(NCHUNK):
                    nc.vector.bn_stats(bnx[:, c, :],
                                       x_nat[:, tt, c * CH:(c + 1) * CH])
                nc.vector.bn_aggr(agg[:, tt, 0:2], bnx)
                nc.vector.tensor_scalar_add(rpack[:, 2 * tt:2 * tt + 1],
                                            agg[:, tt, 1:2], EPS)

            # ---- skewed tail stage B + stores for the previous supertile --
            if prev is not None:
                prev_outs = emit_tail_b(*p_tail)
                for tt in range(NSUB):
                    nc.sync.dma_start(
                        out_d[prev[4] + tt * P:prev[4] + (tt + 1) * P, :],
                        prev_outs[tt])

            # ---- layer sums: gpsimd folds the leading 4 layers of each
            # half, DVE chains the remaining 9 + the merge ----
            def fold_gp(base):
                g = apool.tile([P, NSUB, H], BF16, name=f"g{base}", tag="acc")
                nc.gpsimd.tensor_tensor(g, lay[base], lay[base + 1], ALU.add)
                for li in range(base + 2, base + 5):
                    nc.gpsimd.tensor_tensor(g, g, lay[li], ALU.add)
                return g

            def chain_dv(g, base):
                d = apool.tile([P, NSUB, H], BF16, name=f"d{base}", tag="acc")
                nc.vector.tensor_tensor(d, lay[base], lay[base + 1], ALU.add)
                for li in range(base + 2, base + 8):
                    nc.vector.tensor_tensor(d, d, lay[li], ALU.add)
                nc.vector.tensor_tensor(d, d, g, ALU.add)
                return d

            g1 = fold_gp(0)
            d1 = chain_dv(g1, 5)   # s1 = layers 0..12 summed

            catf = cfpool.tile([P, NSUB, 2 * MO, P], BF16, tag="catf")
            for tt in range(NSUB):
                nc.scalar.dma_start_transpose(catf[:, tt, 0:MO, :],
                                              d1[:, tt, :])

            g2 = fold_gp(13)
            d2 = chain_dv(g2, 18)  # s2 = layers 13..25 summed

            for tt in range(NSUB):
                nc.scalar.dma_start_transpose(catf[:, tt, MO:2 * MO, :],
                                              d2[:, tt, :])

            # ---- GEMM1: 6 m-tiles accumulate k-eagerly (matmuls fire as
            # each cat^T k-group lands: x, then f1, then f2); the last 3
            # m-tiles sweep all 27 k afterwards, reusing banks freed by the
            # eager wave's GELU. PSUM groups are per-2KB-bank, so at most
            # 6 + 3(reused) + 2(GEMM2) banks are ever live. ----
            psA = [psapool.tile([P, TSUP], F32, tag="psA", name=f"psA{j}")
                   for j in range(6)]

            def rhs_k(kko):
                if kko < MO:
                    return catx[:, :, kko, :]
                return catf[:, :, kko - MO, :]

            def g1_matmul(ps, kko, mm):
                nc.tensor.matmul(
                    ps.rearrange("p (a b) -> p a b", a=NSUB),
                    lhsT=w1t_sb[:, kko, mm * P:(mm + 1) * P],
                    rhs=rhs_k(kko),
                    start=(kko == 0),
                    stop=(kko == KO1 - 1),
                )

            for kko in range(KO1):          # eager wave: m = 0..5
                for mm in range(6):
                    g1_matmul(psA[mm], kko, mm)

            gT = gpool.tile([P, MO, TSUP], BF16, tag="gT")
            for mm in range(6):
                nc.scalar.activation(gT[:, mm, :], psA[mm], act,
                                     bias=b1_sb[:, mm:mm + 1])

            psB = [psapool.tile([P, TSUP], F32, tag="psA", name=f"psB{j}")
                   for j in range(3)]
            for kko in range(KO1):          # late sweep: m = 6..8
                for j in range(3):
                    g1_matmul(psB[j], kko, 6 + j)
            for j in range(3):
                nc.scalar.activation(gT[:, 6 + j, :], psB[j], act,
                                     bias=b1_sb[:, 6 + j:7 + j])

            # ---- GEMM2 (+b2 via all-ones stationary x [b2;0..] k-tile) ----
            fgs = []
            for tt in range(NSUB):
                fg = fgpool.tile([P, H], BF16, tag="fg")
                fgs.append(fg)
                for nn in range(NCHUNK):
                    ps2 = ps2pool.tile([P, 512], F32, tag="ps2")
                    for kk in range(KO2):
                        nc.tensor.matmul(
                            ps2[:, 0:CH],
                            lhsT=gT[:, kk, tt * P:(tt + 1) * P],
                            rhs=w2t_sb[:, kk, nn * CH:(nn + 1) * CH],
                            start=(kk == 0),
                            stop=False,
                        )
                    nc.tensor.matmul(
                        ps2[:, 0:CH],
                        lhsT=ones_sb[:, tt * P:(tt + 1) * P],
                        rhs=w2t_sb[:, KO2, nn * CH:(nn + 1) * CH],
                        start=False,
                        stop=True,
                    )
                    nc.scalar.activation(fg[:, nn * CH:(nn + 1) * CH],
                                         ps2[:, 0:CH], AF.Copy)

                # ---- LN2 stats ----
                bnf = spool.tile([P, 3, 6], F32, tag="bnf")
                for c in range(NCHUNK):
                    nc.vector.bn_stats(bnf[:, c, :],
                                       fgs[tt][:, c * CH:(c + 1) * CH])
                nc.vector.bn_aggr(agg[:, tt, 2:4], bnf)
                nc.vector.tensor_scalar_add(rpack[:, 2 * tt + 1:2 * tt + 2],
                                            agg[:, tt, 3:4], EPS)

            prev = (x_nat, fgs, agg, rpack, t0)

        # flush the last supertile's tail
        prev_outs = emit_tail_b(*emit_tail_a(prev))
        for tt in range(NSUB):
            nc.sync.dma_start(
                out_d[prev[4] + tt * P:prev[4] + (tt + 1) * P, :],
                prev_outs[tt])


_NC_CACHE = {}


def _get_nc():
    if "nc" not in _NC_CACHE:
        _NC_CACHE["nc"] = _build_program()
    return _NC_CACHE["nc"]


def _host_prep(codes1, scales1, b1, codes2, scales2, b2,
               ln1_g, ln1_b, ln2_g, ln2_b, alpha):
    # W1 with 1/13 folded into the f1/f2 column blocks (mean -> sum)
    w1 = _dequant_nf4(codes1, scales1)
    # match reference rounding: dequant result is cast to bf16 first
    w1 = w1.astype(NP_BF16).astype(np.float32)
    w1[:, H:] *= np.float32(1.0 / 13.0)
    w1t = np.ascontiguousarray(w1.T).astype(NP_BF16)

    w2 = _dequant_nf4(codes2, scales2).astype(NP_BF16)
    w2t = np.ascontiguousarray(w2.astype(np.float32).T).astype(NP_BF16)
    # extended with the b2 row (k-tile 9 row 0) for the GEMM2 bias fold
    w2te = np.zeros(((KO2 + 1) * P, H), dtype=NP_BF16)
    w2te[:H] = w2t
    w2te[H] = b2.astype(NP_BF16)

    b1s = np.ascontiguousarray(
        b1.astype(np.float32).reshape(MO, P).T)  # [P, MO]

    a32 = alpha.astype(np.float32)
    w_gate = (1.0 / (1.0 + np.exp(-a32[0]))).astype(NP_BF16)
    one_minus = (NP_BF16(1.0) - w_gate)
    g1 = (one_minus.astype(np.float32) * ln1_g.astype(np.float32))
    g2 = (w_gate.astype(np.float32) * ln2_g.astype(np.float32))
    bc = (w_gate.astype(np.float32) * ln2_b.astype(np.float32)
          + one_minus.astype(np.float32) * ln1_b.astype(np.float32))
    g1b = np.ascontiguousarray(np.broadcast_to(g1.astype(NP_BF16), (P, H)))
    g2b = np.ascontiguousarray(np.broadcast_to(g2.astype(NP_BF16), (P, H)))
    bcb = np.ascontiguousarray(np.broadcast_to(bc.astype(NP_BF16), (P, H)))
    return w1t, w2te, b1s, g1b, g2b, bcb


def make_in_maps(image_features, hidden, codes1, scales1, b1, codes2, scales2,
                 b2, ln1_g, ln1_b, ln2_g, ln2_b, alpha):
    w1t, w2te, b1s, g1b, g2b, bcb = _host_prep(
        codes1, scales1, b1, codes2, scales2, b2,
        ln1_g, ln1_b, ln2_g, ln2_b, alpha)
    B = image_features.shape[0]
    in_maps = []
    for c in range(B):
        in_maps.append({
            "x": np.ascontiguousarray(image_features[c]).astype(NP_BF16, copy=False),
            "hid": np.ascontiguousarray(hidden[:, c]).astype(NP_BF16, copy=False),
            "w1t": w1t, "w2t": w2te, "b1s": b1s,
            "g1b": g1b, "g2b": g2b, "bcb": bcb,
        })
    return in_maps


def kernel(image_features, hidden, codes1, scales1, b1, codes2, scales2, b2,
           ln1_g, ln1_b, ln2_g, ln2_b, alpha, _trace=False):
    B, N, Hin = image_features.shape
    assert (B, N, Hin) == (8, NT, H), (B, N, Hin)
    nc = _get_nc()
    in_maps = make_in_maps(image_features, hidden, codes1, scales1, b1,
                           codes2, scales2, b2, ln1_g, ln1_b, ln2_g, ln2_b,
                           alpha)
    res = bass_utils.run_bass_kernel_spmd(
        nc, in_maps, core_ids=list(range(8)), trace=_trace)
    out = np.stack([res.results[c]["out"] for c in range(8)])
    if _trace:
        kernel._last_results = res
    return out.astype(image_features.dtype, copy=False)


# revision 25
# speedup vs baseline: 1.0359x; 1.0359x over previous
"""Trainium2 Bass kernel for nn_Connector_77738908057780 (dense_mlp).

Computation (see reference):
  x   = image_features                      [B, N, H]    bf16
  f1  = mean(hidden[0:13],  axis=0)         [B, N, H]
  f2  = mean(hidden[13:26], axis=0)         [B, N, H]
  cat = concat([x, f1, f2], -1)             [B, N, 3H]
  h   = gelu(cat @ W1.T + b1)               W1 = nf4_dequant(codes1, scales1) [H, 3H]
  fg  = h @ W2.T + b2                       W2 = nf4_dequant(codes2, scales2) [H, H]
  out = w * LN(fg) + (1-w) * LN(x),         w = sigmoid(alpha)

Sharding: data-parallel over batch B=8 -> one batch element per NeuronCore.

v2 design (vs the 286us baseline):
  - skewed software pipeline: supertile st's LN/gate/store stage is emitted
    during st+1 so the DVE never idles waiting on GEMM2.
  - hidden streamed as 13 layer-PAIR DMAs per supertile (1.18 MB each), all
    on the sync HWDGE queue; transposes/weights/consts ride the scalar queue
    so neither blocks the other (separate FIFO rows, SDMA round-robins).
  - the 26-layer sums are split DVE (fast chains) + GpSimd (leading pair
    folds) so DVE load drops from ~155us to ~110us and the DMA never stalls
    on a consumer.
  - GEMM1 is k-eager: all 9 m-tiles accumulate in 4.5 PSUM banks and the
    per-k matmuls fire as soon as each cat^T k-group lands (x first, then
    f1, then f2) - keeps the PE warm (HAM throttle) and off the tail.
  - b2 is folded into GEMM2 as a 10th k-tile (all-ones stationary column x
    a [b2; 0...] row block), killing the DVE bias adds.
  - NF4 dequant of the (small, replicated) weights is host-side weight prep.
"""

import os
import sys

import numpy as np
import ml_dtypes

for _p in ("/opt/trn_rl_repo", "/root/.axon_site/_ro/trn_rl_repo"):
    if os.path.isdir(_p) and _p not in sys.path:
        sys.path.insert(0, _p)

import concourse.bass as bass
import concourse.mybir as mybir
import concourse.tile as tile
from concourse import bacc
from concourse import bass_utils

BF16 = mybir.dt.bfloat16
F32 = mybir.dt.float32
AF = mybir.ActivationFunctionType
ALU = mybir.AluOpType

NP_BF16 = ml_dtypes.bfloat16

P = 128
H = 1152
H3 = 3456
NT = 729          # tokens per core (N); B=8 cores
L = 26
KO1 = H3 // P     # 27 k-tiles for GEMM1
KO2 = H // P      # 9 k-tiles for GEMM2 (+1 ones-tile for the b2 fold)
MO = H // P       # 9 output-feature tiles
EPS = 1e-5
NCHUNK = 3        # fg free-dim chunks of 384
CH = H // NCHUNK  # 384

# Supertiles of exactly 256 tokens; the last overlaps the previous by 39
# tokens (473..511 computed twice, identical values stored twice) so that
# every DMA/compute tile is a full 128-partition tile (729 is not a
# multiple of 128; partial-partition tiles hit HW-hostile DMA paths).
SUPERTILES = [0, 256, 473]
TSUP = 256        # tokens per supertile
NSUB = 2          # 128-token subtiles per supertile

NF4_CODEBOOK = np.array([
    -1.0, -0.6961928009986877, -0.5250730514526367, -0.39491748809814453,
    -0.28444138169288635, -0.18477343022823334, -0.09105003625154495, 0.0,
    0.07958029955625534, 0.16093020141124725, 0.24611230194568634,
    0.33791524171829224, 0.4407098591327667, 0.5626170039176941,
    0.7229568362236023, 1.0], dtype=np.float32)

BLOCK = 64


def _dequant_nf4(codes, scales):
    """Match reference: codebook lookup * per-64-block absmax, cast bf16."""
    out_f, in_f = codes.shape
    w = NF4_CODEBOOK[codes].reshape(out_f, in_f // BLOCK, BLOCK)
    w = w * scales[:, :, None].astype(np.float32)
    return w.reshape(out_f, in_f)  # float32 (caller casts)


def _build_program(act=AF.Gelu):
    nc = bacc.Bacc(
        "TRN2",
        target_bir_lowering=False,
        debug=False,
        num_devices=1,
    )
    x_d = nc.dram_tensor("x", (NT, H), BF16, kind="ExternalInput").ap()
    hid_d = nc.dram_tensor("hid", (L, NT, H), BF16, kind="ExternalInput").ap()
    w1t_d = nc.dram_tensor("w1t", (H3, H), BF16, kind="ExternalInput").ap()
    w2t_d = nc.dram_tensor("w2t", ((KO2 + 1) * P, H), BF16,
                           kind="ExternalInput").ap()
    b1s_d = nc.dram_tensor("b1s", (P, MO), F32, kind="ExternalInput").ap()
    g1b_d = nc.dram_tensor("g1b", (P, H), BF16, kind="ExternalInput").ap()
    g2b_d = nc.dram_tensor("g2b", (P, H), BF16, kind="ExternalInput").ap()
    bcb_d = nc.dram_tensor("bcb", (P, H), BF16, kind="ExternalInput").ap()
    out_d = nc.dram_tensor("out", (NT, H), BF16, kind="ExternalOutput").ap()

    with tile.TileContext(nc) as tc:
        _program(nc, tc, x_d, hid_d, w1t_d, w2t_d, b1s_d,
                 g1b_d, g2b_d, bcb_d, out_d, act)

    nc.compile()
    return nc


def _program(nc, tc, x_d, hid_d, w1t_d, w2t_d, b1s_d, g1b_d, g2b_d,
             bcb_d, out_d, act=AF.Gelu):
    with (
        tc.tile_pool(name="consts", bufs=1) as cpool,
        tc.tile_pool(name="hp", bufs=8) as hpool,
        tc.tile_pool(name="acc", bufs=4) as apool,
        tc.tile_pool(name="catx", bufs=2) as cxpool,
        tc.tile_pool(name="catf", bufs=1) as cfpool,
        tc.tile_pool(name="gt", bufs=1) as gpool,
        tc.tile_pool(name="xn", bufs=2) as xpool,
        tc.tile_pool(name="fg", bufs=4) as fgpool,
        tc.tile_pool(name="outp", bufs=2) as opool,
        tc.tile_pool(name="stats", bufs=2) as spool,
        tc.tile_pool(name="tmp", bufs=1) as tpool,
        tc.tile_pool(name="psA", bufs=6, space="PSUM") as psapool,
        tc.tile_pool(name="ps2", bufs=2, space="PSUM") as ps2pool,
    ):
        # ---- constants ----
        ones_sb = cpool.tile([P, TSUP], BF16)
        nc.gpsimd.memset(ones_sb[:, :], 1.0)
        b1_sb = cpool.tile([P, MO], F32)
        g1b_sb = cpool.tile([P, H], BF16)
        g2b_sb = cpool.tile([P, H], BF16)
        bcb_sb = cpool.tile([P, H], BF16)
        w1t_sb = cpool.tile([P, KO1, H], BF16)
        w2t_sb = cpool.tile([P, KO2 + 1, H], BF16)

        w1t_r = w1t_d.rearrange("(ko p) n -> p ko n", p=P)
        w2t_r = w2t_d.rearrange("(ko p) n -> p ko n", p=P)

        # scalar (ACT) queue: small consts, then x-transposes (emitted in
        # the loop), then the weight chunks.
        nc.scalar.dma_start(b1_sb, b1s_d)
        nc.scalar.dma_start(g1b_sb, g1b_d)
        nc.scalar.dma_start(g2b_sb, g2b_d)
        nc.scalar.dma_start(bcb_sb, bcb_d)

        # state carried between supertiles for the skewed tail stage
        prev = None

        def emit_tail_a(pv):
            """Skewed LN stage A: rsqrt, then the per-token normalizations
            xn = (v - mu) * rsqrt as ONE ACT op each (Identity with
            scale=rsqrt, bias=-mu*rsqrt, both [P,1] APs). Identity/Copy live
            in every activation table, so this adds no table loads."""
            (p_x, p_fgs, p_agg, p_rpack, p_t0) = pv
            ig = spool.tile([P, 2 * NSUB], F32, tag="ig")
            nc.vector.reciprocal(ig, p_rpack)
            nc.scalar.activation(ig, ig, AF.Sqrt)
            bv = spool.tile([P, 2 * NSUB], F32, tag="bv")
            xns = []
            for tt in range(NSUB):
                for ln in range(2):
                    s = 2 * tt + ln
                    # bv = -(mu * ig)
                    nc.vector.tensor_scalar(
                        bv[:, s:s + 1], p_agg[:, tt, 2 * ln:2 * ln + 1],
                        ig[:, s:s + 1], -1.0, ALU.mult, ALU.mult)
                xn1 = tpool.tile([P, H], BF16, tag="xn1")
                nc.scalar.activation(xn1, p_x[:, tt, :], AF.Identity,
                                     bias=bv[:, 2 * tt:2 * tt + 1],
                                     scale=ig[:, 2 * tt:2 * tt + 1])
                xn2 = tpool.tile([P, H], BF16, tag="xn2")
                nc.scalar.activation(xn2, p_fgs[tt], AF.Identity,
                                     bias=bv[:, 2 * tt + 1:2 * tt + 2],
                                     scale=ig[:, 2 * tt + 1:2 * tt + 2])
                xns.append((xn1, xn2))
            return xns

        def emit_tail_b(xns):
            """Skewed LN stage B (DVE, all-bf16 2x-mode tensor_tensor):
            out = xn1 * G1 + xn2 * G2 + Bc."""
            outs = []
            for tt in range(NSUB):
                xn1, xn2 = xns[tt]
                nc.vector.tensor_tensor(xn1, xn1, g1b_sb, ALU.mult)
                nc.vector.tensor_tensor(xn2, xn2, g2b_sb, ALU.mult)
                nc.vector.tensor_tensor(xn1, xn1, xn2, ALU.add)
                out_t = opool.tile([P, H], BF16, tag="outt")
                nc.vector.tensor_tensor(out_t, xn1, bcb_sb, ALU.add)
                outs.append(out_t)
            return outs

        for st_idx, t0 in enumerate(SUPERTILES):
            # ---- DMA issue: x + 26 per-layer loads split across queues ----
            # SP gets x + layers 0-6/13-19 (the early half of each chain);
            # ACT gets layers 7-12/20-25 plus the transposes, so neither
            # queue's in-order sequencer blocks the other's loads.
            x_nat = xpool.tile([P, NSUB, H], BF16, tag="xnat")
            nc.sync.dma_start(
                x_nat,
                x_d[t0:t0 + TSUP, :].rearrange("(s p) f -> p s f", p=P),
            )
            catx = cxpool.tile([P, NSUB, MO, P], BF16, tag="catx")
            for tt in range(NSUB):
                nc.scalar.dma_start_transpose(catx[:, tt, :, :],
                                              x_nat[:, tt, :])

            def load_layer(li, eng):
                lt = hpool.tile([P, NSUB, H], BF16, name=f"hl{li}", tag="hp")
                eng.dma_start(
                    lt,
                    hid_d[li, t0:t0 + TSUP, :].rearrange(
                        "(s p) f -> p s f", p=P))
                return lt

            lay = [None] * L
            sp_layers = list(range(0, 7)) + list(range(13, 20))
            act_layers = list(range(7, 13)) + list(range(20, 26))
            for li in sp_layers:
                lay[li] = load_layer(li, nc.sync)
            if st_idx == 0:
                # weight chunks interleave with the ACT-queue layer loads
                nc.scalar.dma_start(w1t_sb[:, 0:9, :], w1t_r[:, 0:9, :])
            for j, li in enumerate(act_layers):
                lay[li] = load_layer(li, nc.scalar)
                if st_idx == 0 and j in (2, 4, 6):
                    c0 = 9 * (j // 2)
                    if c0 < KO1:
                        nc.scalar.dma_start(w1t_sb[:, c0:c0 + 9, :],
                                            w1t_r[:, c0:c0 + 9, :])
                    else:
                        nc.scalar.dma_start(w2t_sb, w2t_r)

            # ---- skewed tail stage A for the previous supertile ----
            if prev is not None:
                p_tail = emit_tail_a(prev)

            # ---- DVE: x LN stats early (x lands first) ----
            agg = spool.tile([P, NSUB, 4], F32, tag="agg")
            rpack = spool.tile([P, 2 * NSUB], F32, tag="rpack")
            for tt in range(NSUB):
                bnx = spool.tile([P, 3, 6], F32, tag="bnx")
                for c in range(NCHUNK):
                    nc.vector.bn_stats(bnx[:, c, :],
                                       x_nat[:, tt, c * CH:(c + 1) * CH])
                nc.vector.bn_aggr(agg[:, tt, 0:2], bnx)
                nc.vector.tensor_scalar_add(rpack[:, 2 * tt:2 * tt + 1],
                                            agg[:, tt, 1:2], EPS)

            # ---- skewed tail stage B + stores for the previous supertile --
            if prev is not None:
                prev_outs = emit_tail_b(p_tail)
                for tt in range(NSUB):
                    nc.sync.dma_start(
                        out_d[prev[4] + tt * P:prev[4] + (tt + 1) * P, :],
                        prev_outs[tt])

            # ---- layer sums. gpsimd (Add runs at 0.42x roofline, ~4.8us/op)
            # gets only a short 4-layer chain per half, started on the
            # earliest arrivals so it finishes well before it is needed; DVE
            # chains the other 9 layers at arrival rate and folds the gpsimd
            # partial in mid-chain so the merge never gates s1/s2. ----
            def fold_gp(base, n):
                g = apool.tile([P, NSUB, H], BF16, name=f"g{base}", tag="acc")
                nc.gpsimd.tensor_tensor(g, lay[base], lay[base + 1], ALU.add)
                for li in range(base + 2, base + n):
                    nc.gpsimd.tensor_tensor(g, g, lay[li], ALU.add)
                return g

            def chain_dv(g, base, n, merge_after):
                d = apool.tile([P, NSUB, H], BF16, name=f"d{base}", tag="acc")
                nc.vector.tensor_tensor(d, lay[base], lay[base + 1], ALU.add)
                for li in range(base + 2, base + n):
                    nc.vector.tensor_tensor(d, d, lay[li], ALU.add)
                    if li == merge_after:
                        nc.vector.tensor_tensor(d, d, g, ALU.add)
                return d

            g1 = fold_gp(0, 4)
            d1 = chain_dv(g1, 4, 9, merge_after=8)    # s1 = sum(l 0..12)

            catf = cfpool.tile([P, NSUB, 2 * MO, P], BF16, tag="catf")
            for tt in range(NSUB):
                nc.scalar.dma_start_transpose(catf[:, tt, 0:MO, :],
                                              d1[:, tt, :])

            g2 = fold_gp(13, 3)
            d2 = chain_dv(g2, 16, 10, merge_after=22)  # s2 = sum(l 13..25)

            for tt in range(NSUB):
                nc.scalar.dma_start_transpose(catf[:, tt, MO:2 * MO, :],
                                              d2[:, tt, :])

            # ---- GEMM1: 6 m-tiles accumulate k-eagerly (matmuls fire as
            # each cat^T k-group lands: x, then f1, then f2); the last 3
            # m-tiles sweep all 27 k afterwards, reusing banks freed by the
            # eager wave's GELU. PSUM groups are per-2KB-bank, so at most
            # 6 + 3(reused) + 2(GEMM2) banks are ever live. ----
            psA = [psapool.tile([P, TSUP], F32, tag="psA", name=f"psA{j}")
                   for j in range(6)]

            def rhs_k(kko):
                if kko < MO:
                    return catx[:, :, kko, :]
                return catf[:, :, kko - MO, :]

            def g1_matmul(ps, kko, mm):
                nc.tensor.matmul(
                    ps.rearrange("p (a b) -> p a b", a=NSUB),
                    lhsT=w1t_sb[:, kko, mm * P:(mm + 1) * P],
                    rhs=rhs_k(kko),
                    start=(kko == 0),
                    stop=(kko == KO1 - 1),
                )

            for kko in range(KO1):          # eager wave: m = 0..5
                for mm in range(6):
                    g1_matmul(psA[mm], kko, mm)

            gT = gpool.tile([P, MO, TSUP], BF16, tag="gT")
            for mm in range(6):
                nc.scalar.activation(gT[:, mm, :], psA[mm], act,
                                     bias=b1_sb[:, mm:mm + 1])

            psB = [psapool.tile([P, TSUP], F32, tag="psA", name=f"psB{j}")
                   for j in range(3)]
            for kko in range(KO1):          # late sweep: m = 6..8
                for j in range(3):
                    g1_matmul(psB[j], kko, 6 + j)
            for j in range(3):
                nc.scalar.activation(gT[:, 6 + j, :], psB[j], act,
                                     bias=b1_sb[:, 6 + j:7 + j])

            # ---- GEMM2 (+b2 via all-ones stationary x [b2;0..] k-tile) ----
            fgs = []
            for tt in range(NSUB):
                fg = fgpool.tile([P, H], BF16, tag="fg")
                fgs.append(fg)
                for nn in range(NCHUNK):
                    ps2 = ps2pool.tile([P, 512], F32, tag="ps2")
                    for kk in range(KO2):
                        nc.tensor.matmul(
                            ps2[:, 0:CH],
                            lhsT=gT[:, kk, tt * P:(tt + 1) * P],
                            rhs=w2t_sb[:, kk, nn * CH:(nn + 1) * CH],
                            start=(kk == 0),
                            stop=False,
                        )
                    nc.tensor.matmul(
                        ps2[:, 0:CH],
                        lhsT=ones_sb[:, tt * P:(tt + 1) * P],
                        rhs=w2t_sb[:, KO2, nn * CH:(nn + 1) * CH],
                        start=False,
                        stop=True,
                    )
                    nc.scalar.activation(fg[:, nn * CH:(nn + 1) * CH],
                                         ps2[:, 0:CH], AF.Copy)

                # ---- LN2 stats ----
                bnf = spool.tile([P, 3, 6], F32, tag="bnf")
                for c in range(NCHUNK):
                    nc.vector.bn_stats(bnf[:, c, :],
                                       fgs[tt][:, c * CH:(c + 1) * CH])
                nc.vector.bn_aggr(agg[:, tt, 2:4], bnf)
                nc.vector.tensor_scalar_add(rpack[:, 2 * tt + 1:2 * tt + 2],
                                            agg[:, tt, 3:4], EPS)

            prev = (x_nat, fgs, agg, rpack, t0)

        # flush the last supertile's tail
        prev_outs = emit_tail_b(emit_tail_a(prev))
        for tt in range(NSUB):
            nc.sync.dma_start(
                out_d[prev[4] + tt * P:prev[4] + (tt + 1) * P, :],
                prev_outs[tt])


_NC_CACHE = {}


def _get_nc():
    if "nc" not in _NC_CACHE:
        _NC_CACHE["nc"] = _build_program()
    return _NC_CACHE["nc"]


def _host_prep(codes1, scales1, b1, codes2, scales2, b2,
               ln1_g, ln1_b, ln2_g, ln2_b, alpha):
    # W1 with 1/13 folded into the f1/f2 column blocks (mean -> sum)
    w1 = _dequant_nf4(codes1, scales1)
    # match reference rounding: dequant result is cast to bf16 first
    w1 = w1.astype(NP_BF16).astype(np.float32)
    w1[:, H:] *= np.float32(1.0 / 13.0)
    w1t = np.ascontiguousarray(w1.T).astype(NP_BF16)

    w2 = _dequant_nf4(codes2, scales2).astype(NP_BF16)
    w2t = np.ascontiguousarray(w2.astype(np.float32).T).astype(NP_BF16)
    # extended with the b2 row (k-tile 9 row 0) for the GEMM2 bias fold
    w2te = np.zeros(((KO2 + 1) * P, H), dtype=NP_BF16)
    w2te[:H] = w2t
    w2te[H] = b2.astype(NP_BF16)

    b1s = np.ascontiguousarray(
        b1.astype(np.float32).reshape(MO, P).T)  # [P, MO]

    a32 = alpha.astype(np.float32)
    w_gate = (1.0 / (1.0 + np.exp(-a32[0]))).astype(NP_BF16)
    one_minus = (NP_BF16(1.0) - w_gate)
    g1 = (one_minus.astype(np.float32) * ln1_g.astype(np.float32))
    g2 = (w_gate.astype(np.float32) * ln2_g.astype(np.float32))
    bc = (w_gate.astype(np.float32) * ln2_b.astype(np.float32)
          + one_minus.astype(np.float32) * ln1_b.astype(np.float32))
    g1b = np.ascontiguousarray(np.broadcast_to(g1.astype(NP_BF16), (P, H)))
    g2b = np.ascontiguousarray(np.broadcast_to(g2.astype(NP_BF16), (P, H)))
    bcb = np.ascontiguousarray(np.broadcast_to(bc.astype(NP_BF16), (P, H)))
    return w1t, w2te, b1s, g1b, g2b, bcb


def make_in_maps(image_features, hidden, codes1, scales1, b1, codes2, scales2,
                 b2, ln1_g, ln1_b, ln2_g, ln2_b, alpha):
    w1t, w2te, b1s, g1b, g2b, bcb = _host_prep(
        codes1, scales1, b1, codes2, scales2, b2,
        ln1_g, ln1_b, ln2_g, ln2_b, alpha)
    B = image_features.shape[0]
    in_maps = []
    for c in range(B):
        in_maps.append({
            "x": np.ascontiguousarray(image_features[c]).astype(NP_BF16, copy=False),
            "hid": np.ascontiguousarray(hidden[:, c]).astype(NP_BF16, copy=False),
            "w1t": w1t, "w2t": w2te, "b1s": b1s,
            "g1b": g1b, "g2b": g2b, "bcb": bcb,
        })
    return in_maps


def kernel(image_features, hidden, codes1, scales1, b1, codes2, scales2, b2,
           ln1_g, ln1_b, ln2_g, ln2_b, alpha, _trace=False):
    B, N, Hin = image_features.shape
    assert (B, N, Hin) == (8, NT, H), (B, N, Hin)
    nc = _get_nc()
    in_maps = make_in_maps(image_features, hidden, codes1, scales1, b1,
                           codes2, scales2, b2, ln1_g, ln1_b, ln2_g, ln2_b,
                           alpha)
    res = bass_utils.run_bass_kernel_spmd(
        nc, in_maps, core_ids=list(range(8)), trace=_trace)
    out = np.stack([res.results[c]["out"] for c in range(8)])
    if _trace:
        kernel._last_results = res
    return out.astype(image_features.dtype, copy=False)
